# revision 8
# baseline (speedup 1.0000x reference)
"""Single-head causal attention (B=16, T=2048, C=128, H=64) on 8 trn2 cores.

Data-parallel: batch dim sharded 2-per-core. Each core runs a fused
flash-style attention kernel:
  - x^T built on-chip via PE transposes
  - K^T, Q^T projected in [h, t] layout (weights stationary, fp32r)
  - S^T tiles [s=128, tq<=1024] via K-stationary matmuls; causal mask
    applied as a -400*(p-f) ramp matmul that initializes PSUM
  - exp on ACT (scale=1/8 fused); no max-subtraction (logits ~ N(0,1))
  - O^T accumulated in PSUM over s-tiles with V' = [V | 1] so the
    softmax denominator falls out of column 64
  - PE transpose of O^T + per-row reciprocal scale -> natural layout out
"""

import sys

if "/opt/trn_rl_repo" not in sys.path:
    sys.path.insert(0, "/opt/trn_rl_repo")

import numpy as np
import ml_dtypes

import concourse.bass as bass
import concourse.mybir as mybir
import concourse.tile as tile
from concourse import bacc
from concourse.bass_utils import run_bass_kernel_spmd

# Problem constants (hardcoded per contract)
NCORES = 8
B_FULL, T, C, H = 16, 2048, 128, 64
BPC = B_FULL // NCORES  # batches per core = 2
PH = 1024               # tq phase width (2 PSUM banks)
SCALE = 1.0 / np.sqrt(H)
F32 = mybir.dt.float32
F32R = mybir.dt.float32r
BF16 = mybir.dt.bfloat16
EXP = mybir.ActivationFunctionType.Exp

_CACHED = {}


def _r(ap):
    """bitcast an AP to float32r for fast PE matmuls."""
    return ap.bitcast(F32R)


def build_nc():
    nc = bacc.Bacc("TRN2", target_bir_lowering=False)

    x_d = nc.dram_tensor("x", [BPC, T, C], F32, kind="ExternalInput")
    wkd_d = nc.dram_tensor("wkd", [C, 128], F32, kind="ExternalInput")
    wqd_d = nc.dram_tensor("wqd", [C, 128], F32, kind="ExternalInput")
    wv_d = nc.dram_tensor("wv", [C, H], F32, kind="ExternalInput")
    bq2_d = nc.dram_tensor("bq2", [128, 1], F32, kind="ExternalInput")
    bv_d = nc.dram_tensor("bv", [H, 1], F32, kind="ExternalInput")
    ident_d = nc.dram_tensor("ident", [128, 128], F32, kind="ExternalInput")
    rampa_d = nc.dram_tensor("rampa", [128, 128], BF16, kind="ExternalInput")
    rampb_d = nc.dram_tensor("rampb", [128, 128], BF16, kind="ExternalInput")
    o_d = nc.dram_tensor("o", [BPC, T, H], F32, kind="ExternalOutput")

    NT = T // 128  # 16 t-tiles per batch

    with tile.TileContext(nc) as tc:
        with (
            tc.tile_pool(name="consts", bufs=1) as consts,
            tc.tile_pool(name="xload", bufs=2) as xload,
            tc.tile_pool(name="xt", bufs=2) as xtp,
            tc.tile_pool(name="kt", bufs=2) as ktp,
            tc.tile_pool(name="qt", bufs=2) as qtp,
            tc.tile_pool(name="vt", bufs=2) as vtp,
            tc.tile_pool(name="vp", bufs=2) as vpp,
            tc.tile_pool(name="pt", bufs=3) as ptp,
            tc.tile_pool(name="otsb", bufs=2) as otsbp,
            tc.tile_pool(name="osb", bufs=2) as osbp,
            tc.tile_pool(name="small", bufs=4) as smallp,
            tc.tile_pool(name="aux_ps", bufs=2, space=bass.MemorySpace.PSUM) as auxps,
            tc.tile_pool(name="s_ps", bufs=2, space=bass.MemorySpace.PSUM) as sps,
            tc.tile_pool(name="o_ps", bufs=2, space=bass.MemorySpace.PSUM) as ops,
        ):
            # ---- constants ----
            ident = consts.tile([128, 128], F32)
            nc.sync.dma_start(out=ident[:], in_=ident_d[:])
            wkd_raw = consts.tile([C, 128], F32)
            nc.sync.dma_start(out=wkd_raw[:], in_=wkd_d[:])
            wkd = consts.tile([C, 128], F32R)
            nc.vector.tensor_copy(out=wkd[:], in_=wkd_raw[:])
            wqd_raw = consts.tile([C, 128], F32)
            nc.sync.dma_start(out=wqd_raw[:], in_=wqd_d[:])
            wqd = consts.tile([C, 128], F32R)
            nc.vector.tensor_copy(out=wqd[:], in_=wqd_raw[:])
            wv_raw = consts.tile([C, H], F32)
            nc.sync.dma_start(out=wv_raw[:], in_=wv_d[:])
            wv = consts.tile([C, H], F32R)
            nc.vector.tensor_copy(out=wv[:], in_=wv_raw[:])
            bq2 = consts.tile([128, 1], F32)
            nc.sync.dma_start(out=bq2[:], in_=bq2_d[:])
            bv = consts.tile([H, 1], F32)
            nc.sync.dma_start(out=bv[:], in_=bv_d[:])
            rampa = consts.tile([128, 128], BF16)
            nc.sync.dma_start(out=rampa[:], in_=rampa_d[:])
            rampb = consts.tile([128, 128], BF16)
            nc.sync.dma_start(out=rampb[:], in_=rampb_d[:])
            onescol = consts.tile([128, 1], F32)
            nc.vector.memset(onescol[:], 1.0)

            for b in range(BPC):
                # ================= setup: x^T, K^T, Q^T, V' =================
                x_r = x_d[b].rearrange("(n p) c -> p n c", p=128)  # [128, 16, 128]
                xt = xtp.tile([128, T], F32R)  # x^T for this batch
                for g in range(NT // 4):
                    xl = xload.tile([128, 4, 128], F32)
                    nc.sync.dma_start(out=xl[:], in_=x_r[:, 4 * g : 4 * g + 4, :])
                    xps = auxps.tile([128, 512], F32, tag="aux")
                    for k in range(4):
                        nc.tensor.transpose(
                            xps[:, 128 * k : 128 * k + 128], xl[:, k, :], ident[:]
                        )
                    nc.vector.tensor_copy(
                        out=xt[:, 512 * g : 512 * g + 512], in_=xps[:]
                    )

                kt = ktp.tile([128, T], F32R)   # K^T duplicated in both halves
                qt = qtp.tile([128, T], F32R)   # Q^T duplicated, bias added
                vt = vtp.tile([H, T], F32)     # V^T (bias added)
                for g in range(4):
                    sl = slice(512 * g, 512 * g + 512)
                    kps = auxps.tile([128, 512], F32, tag="aux")
                    nc.tensor.matmul(kps[:], wkd[:], xt[:, sl])
                    nc.vector.tensor_copy(out=kt[:, sl], in_=kps[:])
                    qps = auxps.tile([128, 512], F32, tag="aux")
                    nc.tensor.matmul(qps[:], wqd[:], xt[:, sl])
                    nc.vector.tensor_scalar_add(
                        out=qt[:, sl], in0=qps[:], scalar1=bq2[:]
                    )
                    vps = auxps.tile([128, 512], F32, tag="aux")
                    nc.tensor.matmul(vps[0:H, :], wv[:], xt[:, sl])
                    nc.vector.tensor_scalar_add(
                        out=vt[:, sl], in0=vps[0:H, :], scalar1=bv[:]
                    )

                # V' = [V | 1] in natural [s, h'] layout, per s-tile
                vp = vpp.tile([128, NT, H + 1], F32R)
                nc.vector.tensor_copy(
                    out=vp[:, :, H : H + 1],
                    in_=onescol[:].unsqueeze(1).broadcast_to([128, NT, 1]),
                )
                for g in range(4):
                    vtps = auxps.tile([128, 512], F32, tag="aux")
                    for k in range(4):
                        j = 4 * g + k
                        nc.tensor.transpose(
                            vtps[:, H * k : H * k + H],
                            vt[:, 128 * j : 128 * j + 128],
                            ident[0:H, 0:H],
                        )
                    nc.vector.tensor_copy(
                        out=vp[:, 4 * g : 4 * g + 4, 0:H],
                        in_=vtps[:, 0 : 4 * H].rearrange("p (k h) -> p k h", h=H),
                    )

                # ================= main: per tq-phase =================
                for p in range(2):
                    tq0 = PH * p
                    njt = 8 * p + 8  # s-tiles in this phase
                    # O^T psum accumulators for the 2 tq blocks of this phase
                    oacc = [
                        ops.tile([H + 1, 512], F32, name=f"oacc{i}", tag="oacc")
                        for i in range(2)
                    ]
                    for j in range(njt):
                        diag = 8 * p <= j  # this s-tile's diagonal is in-phase
                        d = 128 * j - tq0 if diag else None  # local diag col
                        spt = sps.tile([128, PH], F32)
                        lo = d if diag else 0
                        # causal ramp init on the diagonal 128 cols
                        if diag:
                            nc.tensor.matmul(
                                spt[:, d : d + 128],
                                rampa[:],
                                rampb[:],
                                start=True,
                                stop=False,
                            )
                        # S^T chunks (<=512 wide, bank-aligned pieces)
                        bounds = []
                        if lo < 512:
                            bounds.append((lo, 512))
                            bounds.append((512, PH))
                        else:
                            bounds.append((lo, PH))
                        ktile = kt[0:H, 128 * j : 128 * j + 128]
                        for (c0, c1) in bounds:
                            contains_ramp = diag and c0 <= d < c1
                            nc.tensor.matmul(
                                spt[:, c0:c1],
                                ktile,
                                qt[0:H, tq0 + c0 : tq0 + c1],
                                start=not contains_ramp,
                                stop=True,
                            )
                        # exp (scale folded); write P^T tile
                        pt = ptp.tile([128, PH], F32R)
                        nc.scalar.activation(
                            out=pt[:, lo:PH],
                            in_=spt[:, lo:PH],
                            func=EXP,
                            scale=float(SCALE),
                        )
                        # PV accumulation into O^T blocks
                        for bl in range(2):
                            b0 = 512 * bl
                            if diag and d >= b0 + 512:
                                continue  # block fully masked for this s-tile
                            c0 = max(lo, b0)
                            nc.tensor.matmul(
                                oacc[bl][:, c0 - b0 : 512],
                                vp[:, j, :],
                                pt[:, c0 : b0 + 512],
                                start=(j == 0),
                                stop=(j == njt - 1),
                            )
                    # ---- normalize + transpose + store the 2 blocks ----
                    for bl in range(2):
                        ot = otsbp.tile([H + 1, 512], F32)
                        nc.vector.tensor_copy(out=ot[:], in_=oacc[bl][:])
                        tp = auxps.tile([128, 512], F32, tag="aux")
                        for k in range(4):
                            nc.tensor.transpose(
                                tp[:, (H + 1) * k : (H + 1) * (k + 1)],
                                ot[:, 128 * k : 128 * k + 128],
                                ident[0 : H + 1, 0 : H + 1],
                            )
                        tp3 = tp[:, 0 : 4 * (H + 1)].rearrange(
                            "p (k e) -> p k e", e=H + 1
                        )
                        dn = smallp.tile([128, 4], F32)
                        nc.vector.tensor_copy(out=dn[:], in_=tp3[:, :, H])
                        rc = smallp.tile([128, 4], F32)
                        nc.vector.reciprocal(out=rc[:], in_=dn[:])
                        osb = osbp.tile([128, 4, H], F32)
                        nc.vector.tensor_mul(
                            osb[:],
                            tp3[:, :, 0:H],
                            rc[:].unsqueeze(2).broadcast_to([128, 4, H]),
                        )
                        kg0 = 8 * p + 4 * bl
                        o_r = o_d[b].rearrange("(k pp) h -> pp k h", pp=128)
                        nc.sync.dma_start(
                            out=o_r[:, kg0 : kg0 + 4, :], in_=osb[:]
                        )

    nc.compile()
    return nc


def _host_inputs(inputs):
    Wk = np.asarray(inputs["Wk"], np.float32)
    Wq = np.asarray(inputs["Wq"], np.float32)
    Wv = np.asarray(inputs["Wv"], np.float32)
    bq = np.asarray(inputs["bq"], np.float32)
    bv = np.asarray(inputs["bv"], np.float32)
    # NOTE: bk is mathematically irrelevant (softmax shift invariance):
    # s[t, s] includes q.bk which is constant over s.
    wkd = np.concatenate([Wk, Wk], axis=1).copy()
    wqd = np.concatenate([Wq, Wq], axis=1).copy()
    bq2 = np.concatenate([bq, bq])[:, None].copy()
    bvc = bv[:, None].copy()
    ident = np.eye(128, dtype=np.float32)
    rampa = np.triu(np.ones((128, 128), np.float32)).astype(ml_dtypes.bfloat16)
    rampb = np.tril(np.full((128, 128), -400.0, np.float32), -1).astype(
        ml_dtypes.bfloat16
    )
    return {
        "wkd": wkd, "wqd": wqd, "wv": Wv.copy(), "bq2": bq2, "bv": bvc,
        "ident": ident, "rampa": rampa, "rampb": rampb,
    }


def kernel(**inputs):
    if "nc" not in _CACHED:
        _CACHED["nc"] = build_nc()
    nc = _CACHED["nc"]
    x = np.ascontiguousarray(np.asarray(inputs["x"], np.float32))
    shared = _host_inputs(inputs)
    in_maps = []
    for c in range(NCORES):
        m = dict(shared)
        m["x"] = np.ascontiguousarray(x[BPC * c : BPC * (c + 1)])
        in_maps.append(m)
    res = run_bass_kernel_spmd(nc, in_maps, list(range(NCORES)))
    out = np.concatenate([r["o"] for r in res.results], axis=0)
    return out.astype(np.float32)


if __name__ == "__main__":
    rng = np.random.default_rng(0)
    ins = {
        "x": rng.standard_normal((B_FULL, T, C), np.float32),
        "Wk": rng.standard_normal((C, H), np.float32) / np.sqrt(C),
        "bk": rng.standard_normal((H,), np.float32) / np.sqrt(C),
        "Wq": rng.standard_normal((C, H), np.float32) / np.sqrt(C),
        "bq": rng.standard_normal((H,), np.float32) / np.sqrt(C),
        "Wv": rng.standard_normal((C, H), np.float32) / np.sqrt(C),
        "bv": rng.standard_normal((H,), np.float32) / np.sqrt(C),
    }
    out = kernel(**ins)
    print("out", out.shape, out.dtype, np.abs(out).max())


# revision 10
# speedup vs baseline: 1.0184x; 1.0184x over previous
"""Single-head causal attention (B=16, T=2048, C=128, H=64) on 8 trn2 cores.

Data-parallel: batch dim sharded 2-per-core. Each core runs a fused
flash-style attention kernel:
  - x^T built on-chip via PE transposes
  - K^T, Q^T projected in [h, t] layout (weights stationary, fp32r)
  - S^T tiles [s=128, tq<=1024] via K-stationary matmuls; causal mask
    applied as a -400*(p-f) ramp matmul that initializes PSUM
  - exp on ACT (scale=1/8 fused); no max-subtraction (logits ~ N(0,1))
  - O^T accumulated in PSUM over s-tiles with V' = [V | 1] so the
    softmax denominator falls out of column 64
  - PE transpose of O^T + per-row reciprocal scale -> natural layout out
"""

import sys

if "/opt/trn_rl_repo" not in sys.path:
    sys.path.insert(0, "/opt/trn_rl_repo")

import numpy as np
import ml_dtypes

import concourse.bass as bass
import concourse.mybir as mybir
import concourse.tile as tile
from concourse import bacc
from concourse.bass_utils import run_bass_kernel_spmd

# Problem constants (hardcoded per contract)
NCORES = 8
B_FULL, T, C, H = 16, 2048, 128, 64
BPC = B_FULL // NCORES  # batches per core = 2
PH = 1024               # tq phase width (2 PSUM banks)
SCALE = 1.0 / np.sqrt(H)
F32 = mybir.dt.float32
F32R = mybir.dt.float32r
BF16 = mybir.dt.bfloat16
EXP = mybir.ActivationFunctionType.Exp

_CACHED = {}


def _r(ap):
    """bitcast an AP to float32r for fast PE matmuls."""
    return ap.bitcast(F32R)


def build_nc():
    nc = bacc.Bacc("TRN2", target_bir_lowering=False)

    x_d = nc.dram_tensor("x", [BPC, T, C], F32, kind="ExternalInput")
    wkd_d = nc.dram_tensor("wkd", [C, 128], F32, kind="ExternalInput")
    wqd_d = nc.dram_tensor("wqd", [C, 128], F32, kind="ExternalInput")
    wv_d = nc.dram_tensor("wv", [C, H], F32, kind="ExternalInput")
    bq2_d = nc.dram_tensor("bq2", [128, 1], F32, kind="ExternalInput")
    bv_d = nc.dram_tensor("bv", [H, 1], F32, kind="ExternalInput")
    ident_d = nc.dram_tensor("ident", [128, 128], F32, kind="ExternalInput")
    rampa_d = nc.dram_tensor("rampa", [128, 128], BF16, kind="ExternalInput")
    rampb_d = nc.dram_tensor("rampb", [128, 128], BF16, kind="ExternalInput")
    o_d = nc.dram_tensor("o", [BPC, T, H], F32, kind="ExternalOutput")

    NT = T // 128  # 16 t-tiles per batch

    with tile.TileContext(nc) as tc:
        with (
            tc.tile_pool(name="consts", bufs=1) as consts,
            tc.tile_pool(name="xload", bufs=2) as xload,
            tc.tile_pool(name="xt", bufs=2) as xtp,
            tc.tile_pool(name="kt", bufs=2) as ktp,
            tc.tile_pool(name="qt", bufs=2) as qtp,
            tc.tile_pool(name="vt", bufs=2) as vtp,
            tc.tile_pool(name="vp", bufs=2) as vpp,
            tc.tile_pool(name="pt", bufs=4) as ptp,
            tc.tile_pool(name="otsb", bufs=2) as otsbp,
            tc.tile_pool(name="osb", bufs=2) as osbp,
            tc.tile_pool(name="small", bufs=4) as smallp,
            tc.tile_pool(name="aux_ps", bufs=2, space=bass.MemorySpace.PSUM) as auxps,
            tc.tile_pool(name="s_ps", bufs=2, space=bass.MemorySpace.PSUM) as sps,
            tc.tile_pool(name="o_ps", bufs=2, space=bass.MemorySpace.PSUM) as ops,
        ):
            # ---- constants ----
            ident = consts.tile([128, 128], F32)
            nc.sync.dma_start(out=ident[:], in_=ident_d[:])
            wkd_raw = consts.tile([C, 128], F32)
            nc.sync.dma_start(out=wkd_raw[:], in_=wkd_d[:])
            wkd = consts.tile([C, 128], F32R)
            nc.vector.tensor_copy(out=wkd[:], in_=wkd_raw[:])
            wqd_raw = consts.tile([C, 128], F32)
            nc.sync.dma_start(out=wqd_raw[:], in_=wqd_d[:])
            wqd = consts.tile([C, 128], F32R)
            nc.vector.tensor_copy(out=wqd[:], in_=wqd_raw[:])
            wv_raw = consts.tile([C, H], F32)
            nc.sync.dma_start(out=wv_raw[:], in_=wv_d[:])
            wv = consts.tile([C, H], F32R)
            nc.vector.tensor_copy(out=wv[:], in_=wv_raw[:])
            bq2 = consts.tile([128, 1], F32)
            nc.sync.dma_start(out=bq2[:], in_=bq2_d[:])
            bv = consts.tile([H, 1], F32)
            nc.sync.dma_start(out=bv[:], in_=bv_d[:])
            rampa = consts.tile([128, 128], BF16)
            nc.sync.dma_start(out=rampa[:], in_=rampa_d[:])
            rampb = consts.tile([128, 128], BF16)
            nc.sync.dma_start(out=rampb[:], in_=rampb_d[:])
            onescol = consts.tile([128, 1], F32)
            nc.vector.memset(onescol[:], 1.0)

            for b in range(BPC):
                # ================= setup: x^T, K^T, Q^T, V' =================
                x_r = x_d[b].rearrange("(n p) c -> p n c", p=128)  # [128, 16, 128]
                xt = xtp.tile([128, T], F32R)  # x^T for this batch
                for g in range(NT // 4):
                    xl = xload.tile([128, 4, 128], F32)
                    nc.sync.dma_start(out=xl[:], in_=x_r[:, 4 * g : 4 * g + 4, :])
                    xps = auxps.tile([128, 512], F32, tag="aux")
                    for k in range(4):
                        nc.tensor.transpose(
                            xps[:, 128 * k : 128 * k + 128], xl[:, k, :], ident[:]
                        )
                    nc.vector.tensor_copy(
                        out=xt[:, 512 * g : 512 * g + 512], in_=xps[:]
                    )

                kt = ktp.tile([128, T], F32R)   # K^T duplicated in both halves
                qt = qtp.tile([128, T], F32R)   # Q^T duplicated, bias added
                vt = vtp.tile([H, T], F32)     # V^T (bias added)
                for g in range(4):
                    sl = slice(512 * g, 512 * g + 512)
                    kps = auxps.tile([128, 512], F32, tag="aux")
                    nc.tensor.matmul(kps[:], wkd[:], xt[:, sl])
                    nc.vector.tensor_copy(out=kt[:, sl], in_=kps[:])
                    qps = auxps.tile([128, 512], F32, tag="aux")
                    nc.tensor.matmul(qps[:], wqd[:], xt[:, sl])
                    nc.vector.tensor_scalar_add(
                        out=qt[:, sl], in0=qps[:], scalar1=bq2[:]
                    )
                    vps = auxps.tile([128, 512], F32, tag="aux")
                    nc.tensor.matmul(vps[0:H, :], wv[:], xt[:, sl])
                    nc.vector.tensor_scalar_add(
                        out=vt[:, sl], in0=vps[0:H, :], scalar1=bv[:]
                    )

                # V' = [V | 1] in natural [s, h'] layout, per s-tile
                vp = vpp.tile([128, NT, H + 1], F32R)
                nc.vector.tensor_copy(
                    out=vp[:, :, H : H + 1],
                    in_=onescol[:].unsqueeze(1).broadcast_to([128, NT, 1]),
                )
                for g in range(4):
                    vtps = auxps.tile([128, 512], F32, tag="aux")
                    for k in range(4):
                        j = 4 * g + k
                        nc.tensor.transpose(
                            vtps[:, H * k : H * k + H],
                            vt[:, 128 * j : 128 * j + 128],
                            ident[0:H, 0:H],
                        )
                    nc.vector.tensor_copy(
                        out=vp[:, 4 * g : 4 * g + 4, 0:H],
                        in_=vtps[:, 0 : 4 * H].rearrange("p (k h) -> p k h", h=H),
                    )

                # ================= main: per tq-phase =================
                for p in range(2):
                    tq0 = PH * p
                    njt = 8 * p + 8  # s-tiles in this phase
                    # O^T psum accumulators for the 2 tq blocks of this phase
                    oacc = [
                        ops.tile([H + 1, 512], F32, name=f"oacc{i}", tag="oacc")
                        for i in range(2)
                    ]
                    pending = []

                    def emit_pv(entries, oacc=oacc, njt=njt):
                        for j, lo, pt in entries:
                            for bl in range(2):
                                b0 = 512 * bl
                                if lo >= b0 + 512:
                                    continue  # block fully masked
                                c0 = max(lo, b0)
                                nc.tensor.matmul(
                                    oacc[bl][:, c0 - b0 : 512],
                                    vp[:, j, :],
                                    pt[:, c0 : b0 + 512],
                                    start=(j == 0),
                                    stop=(j == njt - 1),
                                )
                    for m in range(njt // 2):
                        pj = (2 * m, 2 * m + 1)
                        spts, lops = [], []
                        # ramp inits (full-K, brief) for both s-tiles
                        for j in pj:
                            diag = 8 * p <= j
                            d = 128 * j - tq0 if diag else None
                            spt = sps.tile(
                                [128, PH], F32, name=f"spt{j % 2}", tag="spt"
                            )
                            spts.append(spt)
                            lops.append(d if diag else 0)
                            if diag:
                                nc.tensor.matmul(
                                    spt[:, d : d + 128],
                                    rampa[:],
                                    rampb[:],
                                    start=True,
                                    stop=False,
                                )
                        # S^T chunks: the two s-tiles run in disjoint 64-row
                        # PE groups (base partitions 0 / 64) -> concurrent
                        for (c0, c1) in ((0, 512), (512, PH)):
                            for i, j in enumerate(pj):
                                lo = lops[i]
                                if lo >= c1:
                                    continue
                                cc0 = max(lo, c0)
                                h0 = 64 * i
                                has_ramp = (8 * p <= j) and (c0 <= lo < c1)
                                nc.tensor.matmul(
                                    spts[i][:, cc0:c1],
                                    kt[h0 : h0 + H, 128 * j : 128 * j + 128],
                                    qt[h0 : h0 + H, tq0 + cc0 : tq0 + c1],
                                    start=not has_ramp,
                                    stop=True,
                                )
                        # exp (scale folded); write P^T tiles
                        pts = []
                        for i, j in enumerate(pj):
                            lo = lops[i]
                            pt = ptp.tile(
                                [128, PH], F32R, name=f"pt{j % 2}", tag="pt"
                            )
                            pts.append(pt)
                            nc.scalar.activation(
                                out=pt[:, lo:PH],
                                in_=spts[i][:, lo:PH],
                                func=EXP,
                                scale=float(SCALE),
                            )
                        # PV accumulation deferred one pair-iteration so the
                        # PE never stalls on the current exp
                        pending.append(list(zip(pj, lops, pts)))
                        if len(pending) > 1:
                            emit_pv(pending.pop(0))
                    while pending:
                        emit_pv(pending.pop(0))
                    # ---- normalize + transpose + store the 2 blocks ----
                    for bl in range(2):
                        ot = otsbp.tile([H + 1, 512], F32)
                        nc.vector.tensor_copy(out=ot[:], in_=oacc[bl][:])
                        tp = auxps.tile([128, 512], F32, tag="aux")
                        for k in range(4):
                            nc.tensor.transpose(
                                tp[:, (H + 1) * k : (H + 1) * (k + 1)],
                                ot[:, 128 * k : 128 * k + 128],
                                ident[0 : H + 1, 0 : H + 1],
                            )
                        tp3 = tp[:, 0 : 4 * (H + 1)].rearrange(
                            "p (k e) -> p k e", e=H + 1
                        )
                        dn = smallp.tile([128, 4], F32)
                        nc.vector.tensor_copy(out=dn[:], in_=tp3[:, :, H])
                        rc = smallp.tile([128, 4], F32)
                        nc.vector.reciprocal(out=rc[:], in_=dn[:])
                        osb = osbp.tile([128, 4, H], F32)
                        nc.vector.tensor_mul(
                            osb[:],
                            tp3[:, :, 0:H],
                            rc[:].unsqueeze(2).broadcast_to([128, 4, H]),
                        )
                        kg0 = 8 * p + 4 * bl
                        o_r = o_d[b].rearrange("(k pp) h -> pp k h", pp=128)
                        nc.sync.dma_start(
                            out=o_r[:, kg0 : kg0 + 4, :], in_=osb[:]
                        )

    nc.compile()
    return nc


def _host_inputs(inputs):
    Wk = np.asarray(inputs["Wk"], np.float32)
    Wq = np.asarray(inputs["Wq"], np.float32)
    Wv = np.asarray(inputs["Wv"], np.float32)
    bq = np.asarray(inputs["bq"], np.float32)
    bv = np.asarray(inputs["bv"], np.float32)
    # NOTE: bk is mathematically irrelevant (softmax shift invariance):
    # s[t, s] includes q.bk which is constant over s.
    wkd = np.concatenate([Wk, Wk], axis=1).copy()
    wqd = np.concatenate([Wq, Wq], axis=1).copy()
    bq2 = np.concatenate([bq, bq])[:, None].copy()
    bvc = bv[:, None].copy()
    ident = np.eye(128, dtype=np.float32)
    rampa = np.triu(np.ones((128, 128), np.float32)).astype(ml_dtypes.bfloat16)
    rampb = np.tril(np.full((128, 128), -400.0, np.float32), -1).astype(
        ml_dtypes.bfloat16
    )
    return {
        "wkd": wkd, "wqd": wqd, "wv": Wv.copy(), "bq2": bq2, "bv": bvc,
        "ident": ident, "rampa": rampa, "rampb": rampb,
    }


def kernel(**inputs):
    if "nc" not in _CACHED:
        _CACHED["nc"] = build_nc()
    nc = _CACHED["nc"]
    x = np.ascontiguousarray(np.asarray(inputs["x"], np.float32))
    shared = _host_inputs(inputs)
    in_maps = []
    for c in range(NCORES):
        m = dict(shared)
        m["x"] = np.ascontiguousarray(x[BPC * c : BPC * (c + 1)])
        in_maps.append(m)
    res = run_bass_kernel_spmd(nc, in_maps, list(range(NCORES)))
    out = np.concatenate([r["o"] for r in res.results], axis=0)
    return out.astype(np.float32)


if __name__ == "__main__":
    rng = np.random.default_rng(0)
    ins = {
        "x": rng.standard_normal((B_FULL, T, C), np.float32),
        "Wk": rng.standard_normal((C, H), np.float32) / np.sqrt(C),
        "bk": rng.standard_normal((H,), np.float32) / np.sqrt(C),
        "Wq": rng.standard_normal((C, H), np.float32) / np.sqrt(C),
        "bq": rng.standard_normal((H,), np.float32) / np.sqrt(C),
        "Wv": rng.standard_normal((C, H), np.float32) / np.sqrt(C),
        "bv": rng.standard_normal((H,), np.float32) / np.sqrt(C),
    }
    out = kernel(**ins)
    print("out", out.shape, out.dtype, np.abs(out).max())


# revision 12
# speedup vs baseline: 1.2875x; 1.2642x over previous
"""Single-head causal attention (B=16, T=2048, C=128, H=64) on 8 trn2 cores.

Data-parallel: batch dim sharded 2-per-core. Each core runs a fused
flash-style attention kernel:
  - x^T built on-chip via PE transposes
  - K^T, Q^T projected in [h, t] layout (weights stationary, fp32r)
  - S^T tiles [s=128, tq<=1024] via K-stationary matmuls; causal mask
    applied as a -400*(p-f) ramp matmul that initializes PSUM
  - exp on ACT (scale=1/8 fused); no max-subtraction (logits ~ N(0,1))
  - O^T accumulated in PSUM over s-tiles with V' = [V | 1] so the
    softmax denominator falls out of column 64
  - PE transpose of O^T + per-row reciprocal scale -> natural layout out
"""

import sys

if "/opt/trn_rl_repo" not in sys.path:
    sys.path.insert(0, "/opt/trn_rl_repo")

import numpy as np
import ml_dtypes

import concourse.bass as bass
import concourse.mybir as mybir
import concourse.tile as tile
from concourse import bacc
from concourse.bass_utils import run_bass_kernel_spmd

# Problem constants (hardcoded per contract)
NCORES = 8
B_FULL, T, C, H = 16, 2048, 128, 64
BPC = B_FULL // NCORES  # batches per core = 2
PH = 1024               # tq phase width (2 PSUM banks)
SCALE = 1.0 / np.sqrt(H)
F32 = mybir.dt.float32
F32R = mybir.dt.float32r
BF16 = mybir.dt.bfloat16
EXP = mybir.ActivationFunctionType.Exp

_CACHED = {}


def _r(ap):
    """bitcast an AP to float32r for fast PE matmuls."""
    return ap.bitcast(F32R)


def build_nc():
    nc = bacc.Bacc("TRN2", target_bir_lowering=False)

    x_d = nc.dram_tensor("x", [BPC, T, C], F32, kind="ExternalInput")
    wkd_d = nc.dram_tensor("wkd", [C, 128], BF16, kind="ExternalInput")
    wqd_d = nc.dram_tensor("wqd", [C, 128], BF16, kind="ExternalInput")
    wv_d = nc.dram_tensor("wv", [C, H], BF16, kind="ExternalInput")
    bq2_d = nc.dram_tensor("bq2", [128, 1], F32, kind="ExternalInput")
    bv_d = nc.dram_tensor("bv", [H, 1], F32, kind="ExternalInput")
    ident_d = nc.dram_tensor("ident", [128, 128], F32, kind="ExternalInput")
    identb_d = nc.dram_tensor("identb", [128, 128], BF16, kind="ExternalInput")
    rampa_d = nc.dram_tensor("rampa", [128, 128], BF16, kind="ExternalInput")
    rampb_d = nc.dram_tensor("rampb", [128, 128], BF16, kind="ExternalInput")
    o_d = nc.dram_tensor("o", [BPC, T, H], F32, kind="ExternalOutput")

    NT = T // 128  # 16 t-tiles per batch

    with tile.TileContext(nc) as tc:
        with (
            tc.tile_pool(name="consts", bufs=1) as consts,
            tc.tile_pool(name="xload", bufs=2) as xload,
            tc.tile_pool(name="xt", bufs=2) as xtp,
            tc.tile_pool(name="kt", bufs=2) as ktp,
            tc.tile_pool(name="qt", bufs=2) as qtp,
            tc.tile_pool(name="vt", bufs=2) as vtp,
            tc.tile_pool(name="vp", bufs=2) as vpp,
            tc.tile_pool(name="pt", bufs=4) as ptp,
            tc.tile_pool(name="otsb", bufs=2) as otsbp,
            tc.tile_pool(name="osb", bufs=2) as osbp,
            tc.tile_pool(name="small", bufs=4) as smallp,
            tc.tile_pool(name="aux_ps", bufs=2, space=bass.MemorySpace.PSUM) as auxps,
            tc.tile_pool(name="s_ps", bufs=2, space=bass.MemorySpace.PSUM) as sps,
            tc.tile_pool(name="o_ps", bufs=2, space=bass.MemorySpace.PSUM) as ops,
        ):
            # ---- constants ----
            ident = consts.tile([128, 128], F32)
            nc.sync.dma_start(out=ident[:], in_=ident_d[:])
            wkd = consts.tile([C, 128], BF16)
            nc.sync.dma_start(out=wkd[:], in_=wkd_d[:])
            wqd = consts.tile([C, 128], BF16)
            nc.sync.dma_start(out=wqd[:], in_=wqd_d[:])
            wv = consts.tile([C, H], BF16)
            nc.sync.dma_start(out=wv[:], in_=wv_d[:])
            identb = consts.tile([128, 128], BF16)
            nc.sync.dma_start(out=identb[:], in_=identb_d[:])
            bq2 = consts.tile([128, 1], F32)
            nc.sync.dma_start(out=bq2[:], in_=bq2_d[:])
            bv = consts.tile([H, 1], F32)
            nc.sync.dma_start(out=bv[:], in_=bv_d[:])
            rampa = consts.tile([128, 128], BF16)
            nc.sync.dma_start(out=rampa[:], in_=rampa_d[:])
            rampb = consts.tile([128, 128], BF16)
            nc.sync.dma_start(out=rampb[:], in_=rampb_d[:])
            onescol = consts.tile([128, 1], F32)
            nc.vector.memset(onescol[:], 1.0)

            for b in range(BPC):
                # ================= setup: x^T, K^T, Q^T, V' =================
                x_r = x_d[b].rearrange("(n p) c -> p n c", p=128)  # [128, 16, 128]
                xt = xtp.tile([128, T], BF16)  # x^T for this batch
                for g in range(NT // 4):
                    xl = xload.tile([128, 4, 128], F32)
                    nc.sync.dma_start(out=xl[:], in_=x_r[:, 4 * g : 4 * g + 4, :])
                    xlb = xload.tile([128, 4, 128], BF16)
                    nc.vector.tensor_copy(out=xlb[:], in_=xl[:])
                    xps = auxps.tile([128, 512], BF16, tag="aux")
                    for k in range(4):
                        nc.tensor.transpose(
                            xps[:, 128 * k : 128 * k + 128], xlb[:, k, :], identb[:]
                        )
                    nc.vector.tensor_copy(
                        out=xt[:, 512 * g : 512 * g + 512], in_=xps[:]
                    )

                kt = ktp.tile([128, T], BF16)   # K^T duplicated in both halves
                qt = qtp.tile([128, T], BF16)   # Q^T duplicated, bias added
                vt = vtp.tile([H, T], BF16)     # V^T (bias added)
                for g in range(4):
                    sl = slice(512 * g, 512 * g + 512)
                    kps = auxps.tile([128, 512], F32, tag="aux")
                    nc.tensor.matmul(kps[:], wkd[:], xt[:, sl])
                    nc.vector.tensor_copy(out=kt[:, sl], in_=kps[:])
                    qps = auxps.tile([128, 512], F32, tag="aux")
                    nc.tensor.matmul(qps[:], wqd[:], xt[:, sl])
                    nc.vector.tensor_scalar_add(
                        out=qt[:, sl], in0=qps[:], scalar1=bq2[:]
                    )
                    vps = auxps.tile([128, 512], F32, tag="aux")
                    nc.tensor.matmul(vps[0:H, :], wv[:], xt[:, sl])
                    nc.vector.tensor_scalar_add(
                        out=vt[:, sl], in0=vps[0:H, :], scalar1=bv[:]
                    )

                # V' = [V | 1] in natural [s, h'] layout, per s-tile
                vp = vpp.tile([128, NT, H + 1], BF16)
                nc.vector.tensor_copy(
                    out=vp[:, :, H : H + 1],
                    in_=onescol[:].unsqueeze(1).broadcast_to([128, NT, 1]),
                )
                for g in range(4):
                    vtps = auxps.tile([128, 512], BF16, tag="aux")
                    for k in range(4):
                        j = 4 * g + k
                        nc.tensor.transpose(
                            vtps[:, H * k : H * k + H],
                            vt[:, 128 * j : 128 * j + 128],
                            identb[0:H, 0:H],
                        )
                    nc.vector.tensor_copy(
                        out=vp[:, 4 * g : 4 * g + 4, 0:H],
                        in_=vtps[:, 0 : 4 * H].rearrange("p (k h) -> p k h", h=H),
                    )

                # ================= main: per tq-phase =================
                for p in range(2):
                    tq0 = PH * p
                    njt = 8 * p + 8  # s-tiles in this phase
                    # O^T psum accumulators for the 2 tq blocks of this phase
                    oacc = [
                        ops.tile([H + 1, 512], F32, name=f"oacc{i}", tag="oacc")
                        for i in range(2)
                    ]
                    pending = []

                    def emit_pv(entries, oacc=oacc, njt=njt):
                        for j, lo, pt in entries:
                            for bl in range(2):
                                b0 = 512 * bl
                                if lo >= b0 + 512:
                                    continue  # block fully masked
                                c0 = max(lo, b0)
                                nc.tensor.matmul(
                                    oacc[bl][:, c0 - b0 : 512],
                                    vp[:, j, :],
                                    pt[:, c0 : b0 + 512],
                                    start=(j == 0),
                                    stop=(j == njt - 1),
                                )
                    for m in range(njt // 2):
                        pj = (2 * m, 2 * m + 1)
                        spts, lops = [], []
                        # ramp inits (full-K, brief) for both s-tiles
                        for j in pj:
                            diag = 8 * p <= j
                            d = 128 * j - tq0 if diag else None
                            spt = sps.tile(
                                [128, PH], F32, name=f"spt{j % 2}", tag="spt"
                            )
                            spts.append(spt)
                            lops.append(d if diag else 0)
                            if diag:
                                nc.tensor.matmul(
                                    spt[:, d : d + 128],
                                    rampa[:],
                                    rampb[:],
                                    start=True,
                                    stop=False,
                                )
                        # S^T chunks: the two s-tiles run in disjoint 64-row
                        # PE groups (base partitions 0 / 64) -> concurrent
                        for (c0, c1) in ((0, 512), (512, PH)):
                            for i, j in enumerate(pj):
                                lo = lops[i]
                                if lo >= c1:
                                    continue
                                cc0 = max(lo, c0)
                                h0 = 64 * i
                                has_ramp = (8 * p <= j) and (c0 <= lo < c1)
                                nc.tensor.matmul(
                                    spts[i][:, cc0:c1],
                                    kt[h0 : h0 + H, 128 * j : 128 * j + 128],
                                    qt[h0 : h0 + H, tq0 + cc0 : tq0 + c1],
                                    start=not has_ramp,
                                    stop=True,
                                )
                        # exp (scale folded); write P^T tiles
                        pts = []
                        for i, j in enumerate(pj):
                            lo = lops[i]
                            pt = ptp.tile(
                                [128, PH], BF16, name=f"pt{j % 2}", tag="pt"
                            )
                            pts.append(pt)
                            nc.scalar.activation(
                                out=pt[:, lo:PH],
                                in_=spts[i][:, lo:PH],
                                func=EXP,
                                scale=float(SCALE),
                            )
                        # PV accumulation deferred one pair-iteration so the
                        # PE never stalls on the current exp
                        pending.append(list(zip(pj, lops, pts)))
                        if len(pending) > 1:
                            emit_pv(pending.pop(0))
                    while pending:
                        emit_pv(pending.pop(0))
                    # ---- normalize + transpose + store the 2 blocks ----
                    for bl in range(2):
                        ot = otsbp.tile([H + 1, 512], F32)
                        nc.vector.tensor_copy(out=ot[:], in_=oacc[bl][:])
                        tp = auxps.tile([128, 512], F32, tag="aux")
                        for k in range(4):
                            nc.tensor.transpose(
                                tp[:, (H + 1) * k : (H + 1) * (k + 1)],
                                ot[:, 128 * k : 128 * k + 128],
                                ident[0 : H + 1, 0 : H + 1],
                            )
                        tp3 = tp[:, 0 : 4 * (H + 1)].rearrange(
                            "p (k e) -> p k e", e=H + 1
                        )
                        dn = smallp.tile([128, 4], F32)
                        nc.vector.tensor_copy(out=dn[:], in_=tp3[:, :, H])
                        rc = smallp.tile([128, 4], F32)
                        nc.vector.reciprocal(out=rc[:], in_=dn[:])
                        osb = osbp.tile([128, 4, H], F32)
                        nc.vector.tensor_mul(
                            osb[:],
                            tp3[:, :, 0:H],
                            rc[:].unsqueeze(2).broadcast_to([128, 4, H]),
                        )
                        kg0 = 8 * p + 4 * bl
                        o_r = o_d[b].rearrange("(k pp) h -> pp k h", pp=128)
                        nc.sync.dma_start(
                            out=o_r[:, kg0 : kg0 + 4, :], in_=osb[:]
                        )

    nc.compile()
    return nc


def _host_inputs(inputs):
    Wk = np.asarray(inputs["Wk"], np.float32)
    Wq = np.asarray(inputs["Wq"], np.float32)
    Wv = np.asarray(inputs["Wv"], np.float32)
    bq = np.asarray(inputs["bq"], np.float32)
    bv = np.asarray(inputs["bv"], np.float32)
    # NOTE: bk is mathematically irrelevant (softmax shift invariance):
    # s[t, s] includes q.bk which is constant over s.
    wkd = np.concatenate([Wk, Wk], axis=1).astype(ml_dtypes.bfloat16)
    wqd = np.concatenate([Wq, Wq], axis=1).astype(ml_dtypes.bfloat16)
    bq2 = np.concatenate([bq, bq])[:, None].copy()
    bvc = bv[:, None].copy()
    ident = np.eye(128, dtype=np.float32)
    identb = np.eye(128, dtype=np.float32).astype(ml_dtypes.bfloat16)
    rampa = np.triu(np.ones((128, 128), np.float32)).astype(ml_dtypes.bfloat16)
    rampb = np.tril(np.full((128, 128), -400.0, np.float32), -1).astype(
        ml_dtypes.bfloat16
    )
    return {
        "wkd": wkd, "wqd": wqd, "wv": Wv.astype(ml_dtypes.bfloat16),
        "bq2": bq2, "bv": bvc,
        "ident": ident, "identb": identb, "rampa": rampa, "rampb": rampb,
    }


def kernel(**inputs):
    if "nc" not in _CACHED:
        _CACHED["nc"] = build_nc()
    nc = _CACHED["nc"]
    x = np.ascontiguousarray(np.asarray(inputs["x"], np.float32))
    shared = _host_inputs(inputs)
    in_maps = []
    for c in range(NCORES):
        m = dict(shared)
        m["x"] = np.ascontiguousarray(x[BPC * c : BPC * (c + 1)])
        in_maps.append(m)
    res = run_bass_kernel_spmd(nc, in_maps, list(range(NCORES)))
    out = np.concatenate([r["o"] for r in res.results], axis=0)
    return out.astype(np.float32)


if __name__ == "__main__":
    rng = np.random.default_rng(0)
    ins = {
        "x": rng.standard_normal((B_FULL, T, C), np.float32),
        "Wk": rng.standard_normal((C, H), np.float32) / np.sqrt(C),
        "bk": rng.standard_normal((H,), np.float32) / np.sqrt(C),
        "Wq": rng.standard_normal((C, H), np.float32) / np.sqrt(C),
        "bq": rng.standard_normal((H,), np.float32) / np.sqrt(C),
        "Wv": rng.standard_normal((C, H), np.float32) / np.sqrt(C),
        "bv": rng.standard_normal((H,), np.float32) / np.sqrt(C),
    }
    out = kernel(**ins)
    print("out", out.shape, out.dtype, np.abs(out).max())
